# revision 16
# baseline (speedup 1.0000x reference)
"""SpGAT_Conv Trainium2 kernel: 8-core SPMD spectral GNN conv.

Math (reference):
    a = softmax(alpha)
    pre = x @ W                                   [N, D]
    out_low  = s0 @ (a0 * (s1 @ pre))             [N, D]
    out_high = s2 @ (a1 * (s3 @ pre))             [N, D]
    out = relu(max(out_low, out_high) + bias)

Re-association: t = S @ (x @ W) == (S @ x) @ W with S = concat(s1, s3).
Row-sharding t's rows across 8 cores makes the x@W work perfectly sharded
too (it rides on each core's own 1024 rows of u = S_c @ x) instead of being
replicated, cutting per-core PE work from 1280 to 1056 big matmuls:

    step 1: u_c^T = x^T S_c^T accumulated over n-chunks; stationary = x
            chunks (natural layout), moving = S_c^T strips.  Two sweeps of
            512 i-columns each (4 PSUM banks per sweep).
    step 2: t_c = u_c @ W via u^T slices stationary, W moving (32 matmuls);
            each sweep's 512 t rows are staged + AllGathered in two
            sub-collectives (4 total, order 0,1,3,2) while later compute
            runs.
    phase 3: out_c = relu(max(s0_c@t1 + bias, s2_c@t3 + bias)), high band
             then low band over ARRIVAL-ordered t chunks; softmax(alpha)
             is folded into s0/s2 on the host, bias is passed
             pre-broadcast [128, D] and seeded into PSUM, so the epilogue
             per strip is one ACT relu + one DVE max.  The last four
             low-band strips run nt-major so the 8 accumulators complete
             staggered and the epilogue pipelines behind the matmuls.

Schedule notes:
  - PE warm-up junk matmuls keep the HAM activity window busy from
    ~6.5us so the first real matmuls aren't clock-gated.
  - Steady-state matmul rate is ~263ns (chip P0 power downclock to
    ~2.0GHz under sustained full-chip load; pure-PE microbenches run at
    216ns but PE+DMA+CC tips the power budget).  The schedule aims for a
    gapless PE stream rather than fighting the clock.
  - strips3 bufs=5 gives phase 3 a ~21us strip cushion so the AllGather
    ring-traffic windows (which degrade concurrent DMA supply) don't
    starve the PE.

DMA discipline: DMA_DIRECT2D executes synchronously on the issuing
engine's queue AND all queues share one small DMA-completion semaphore
pool, so a blocked DMA (or a semaphore-reset rendezvous with one) stalls
unrelated queues.  Mitigations:
  - Sync queue: only matmul-pacing strip loads, t_in stores, the out
    stores (tail-only) — never anything collective-gated.
  - x/w bulk loads ride scalar + gpsimd as FEW BIG block DMAs (small
    DMA counts keep semaphore epochs from recycling).
  - AllGather-gated t_sb loads ride the otherwise-idle gpsimd queue.

All big operands are host-cast to bf16 (full PE rate) with fp32 PSUM
accumulation; s-matrices are host-transposed so the contraction dim lands
on SBUF partitions with contiguous DMAs.  x needs no transpose in this
formulation.
"""

import os

import numpy as np

N_CORES = 8
N = 8192
K = 2048
NK = N - K          # 6144
D = 512
ROWS = N // N_CORES  # 1024 rows per core
P = 128
RCH = ROWS // P      # 8  (row chunks per core / output strips)
NCH = N // P         # 64 (contraction chunks over full N)
KCH = K // P         # 16 (low-band chunks; high band = NCH - KCH = 48)
DCH = D // P         # 4  (depth chunks)
NSUB = 4             # sub-AllGathers for t
SUBR = ROWS // NSUB  # 256 rows per rank per sub-AG
SB = 4               # n-chunks per sweep-strip DMA batch
NWARM = 8            # PE warm-up junk matmuls
NTAIL = 4            # low-band strips run nt-major for the epilogue

DEBUG = os.environ.get("SPGAT_DEBUG", "0") == "1"

_CACHE = {}

# t-chunk arrival order: sub-AG g delivers, for every rank c, t rows
# [1024c + 256g, 1024c + 256(g+1)) = global chunks 8c + 2g + {0,1}.
# Gathers are triggered in order 0, 1, 3, 2 (sweep B stages its upper rows
# first), so consume in that order too.  Phase 3 iterates PAIRS (both u's
# of one (g, c)) so each pair is one strip DMA.
GORDER = [0, 1, 3, 2]
PAIRS = [(8 * c + 2 * g, g, c) for g in GORDER for c in range(N_CORES)]


def _build_nc():
    import concourse.mybir as mybir
    import concourse.tile as tile
    from concourse import bacc

    f32 = mybir.dt.float32
    bf16 = mybir.dt.bfloat16
    cdt = bf16

    nc = bacc.Bacc(
        "TRN2", target_bir_lowering=False, debug=False, num_devices=N_CORES
    )

    # all big operands host-packed so every DMA reads 2KB+ contiguous
    # per-partition lines (max descriptor efficiency)
    xp = nc.dram_tensor("xp", [NCH // 2, P, 2 * D], cdt,
                        kind="ExternalInput").ap()
    w = nc.dram_tensor("w", [D, D], cdt, kind="ExternalInput").ap()
    seedb = nc.dram_tensor("seedb", [P, D], f32, kind="ExternalInput").ap()
    stp = nc.dram_tensor("stp", [2, NCH // SB, P, SB * D], cdt,
                         kind="ExternalInput").ap()
    s0p = nc.dram_tensor("s0p", [KCH // 2, P, 2 * ROWS], cdt,
                         kind="ExternalInput").ap()
    s2p = nc.dram_tensor("s2p", [(NCH - KCH) // 2, P, 2 * ROWS], cdt,
                         kind="ExternalInput").ap()
    out = nc.dram_tensor("out", [ROWS, D], f32, kind="ExternalOutput").ap()
    if DEBUG:
        t_dump = nc.dram_tensor("t_dump", [N, D], cdt, kind="ExternalOutput").ap()

    groups = [list(range(N_CORES))]

    with tile.TileContext(nc) as tc:
        with (
            tc.tile_pool(name="const", bufs=1) as const,
            tc.tile_pool(name="bigA", bufs=1) as bigA,
            tc.tile_pool(name="bigB", bufs=1) as bigB,
            tc.tile_pool(name="strips1", bufs=4) as strips1,
            tc.tile_pool(name="strips3", bufs=5) as strips3,
            tc.tile_pool(name="stage", bufs=4) as stage,
            tc.tile_pool(name="epi", bufs=2) as epi,
            tc.tile_pool(name="stash", bufs=1) as stashp,
            tc.tile_pool(name="ps", bufs=8, space="PSUM") as ps,
            tc.tile_pool(name="dram", bufs=1, space="DRAM") as dram,
        ):
            # ---- collective warm-up: absorb CC startup + skew.  No input
            # deps so the trigger is the very first gpsimd instruction.
            warm_in = dram.tile([8, 8], f32, name="warm_in")
            warm_out = dram.tile([64, 8], f32, name="warm_out", addr_space="Shared")
            nc.gpsimd.collective_compute(
                "AllGather",
                mybir.AluOpType.bypass,
                replica_groups=groups,
                ins=[warm_in.opt()],
                outs=[warm_out.opt()],
            )

            # ---- PSUM allocation order fixes the 8-bank rotation; all
            # tiles are one 2KB bank.
            accsA = [ps.tile([P, D], f32, name=f"uA_{dc}", tag="acc")
                     for dc in range(DCH)]
            accsB = [ps.tile([P, D], f32, name=f"uB_{dc}", tag="acc")
                     for dc in range(DCH)]

            # ---- PE warm-up junk matmuls (HAM activity) on memset tiles;
            # they accumulate into accsA[0] with start=True and the first
            # real matmul (j == 0, start=True) resets the bank.
            wj = const.tile([P, P], cdt, name="wj")
            mj = const.tile([P, D], cdt, name="mj")
            nc.vector.memset(wj[:], 0.0)
            nc.vector.memset(mj[:], 0.0)
            for _ in range(NWARM):
                nc.tensor.matmul(accsA[0][:], wj[:], mj[:], start=True, stop=True)

            # ---- input DMAs.  Sync: x chunks 0,1 singly (j=0 matmuls
            # fire asap), then only strip loads.  x bulk rides scalar +
            # gpsimd as few big paced blocks; w + bias seed on scalar.
            x_sb = bigA.tile([P, NCH, D], cdt, name="x_sb", tag="bigA")
            nc.sync.dma_start(x_sb[:, 0, :], xp[0][:, :D])
            nc.sync.dma_start(x_sb[:, 1, :], xp[0][:, D:])
            blocks = [(1, 2), (2, 3), (3, 4), (4, 6), (6, 8), (8, 10),
                      (10, 12), (12, 16), (16, 20), (20, 24), (24, 32)]
            for i, (b0, b1) in enumerate(blocks):
                eng = nc.scalar if i % 2 == 0 else nc.gpsimd
                eng.dma_start(
                    x_sb[:, 2 * b0 : 2 * b1, :].rearrange("p c d -> p (c d)"),
                    xp[b0:b1].rearrange("b p a -> p b a"),
                )
            w_sb = const.tile([P, DCH, D], cdt, name="w_sb")
            nc.scalar.dma_start(w_sb[:], w.rearrange("(c p) d -> p c d", p=P))
            bsb = const.tile([P, D], f32, name="bsb")
            nc.scalar.dma_start(bsb[:], seedb[:])

            # ---- steps 1+2 infrastructure
            t_in = dram.tile([ROWS, D], cdt, name="t_in")
            t_outs = [
                dram.tile([SUBR * N_CORES, D], cdt, name=f"t_out{g}",
                          addr_space="Shared")
                for g in range(NSUB)
            ]

            def t_subag(g):
                nc.gpsimd.collective_compute(
                    "AllGather",
                    mybir.AluOpType.bypass,
                    replica_groups=groups,
                    ins=[t_in[SUBR * g : SUBR * (g + 1), :].opt()],
                    outs=[t_outs[g].opt()],
                )

            ut_sb = [
                const.tile([P, DCH, D], cdt, name=f"ut{sw}") for sw in range(2)
            ]

            def sweep1(sw, accs, b0, b1, split_first=False):
                """step-1 matmuls for strip batches [b0, b1) of sweep sw;
                each batch is SB n-chunks in one contiguous-packed DMA."""
                for bk in range(b0, b1):
                    j0 = SB * bk
                    strip = strips1.tile([P, SB, D], cdt, name=f"s{sw}_{bk}",
                                         tag="strip")
                    if split_first and bk == b0:
                        for j2 in range(SB):
                            nc.sync.dma_start(
                                strip[:, j2, :],
                                stp[sw, bk][:, D * j2 : D * (j2 + 1)],
                            )
                    else:
                        nc.sync.dma_start(
                            strip[:].rearrange("p a b -> p (a b)"),
                            stp[sw, bk],
                        )
                    for j2 in range(SB):
                        j = j0 + j2
                        for dc in range(DCH):
                            nc.tensor.matmul(
                                accs[dc][:],
                                x_sb[:, j, P * dc : P * (dc + 1)],
                                strip[:, j2, :],
                                start=(j == 0),
                                stop=(j == NCH - 1),
                            )

            def drain_u(sw, accs):
                # split across DVE and ACT so the banks free ~2x faster
                for dc in range(2):
                    nc.vector.tensor_copy(ut_sb[sw][:, dc, :], accs[dc][:])
                for dc in range(2, DCH):
                    nc.scalar.copy(ut_sb[sw][:, dc, :], accs[dc][:])

            def step2(sw, tps=None):
                """t rows [512sw, 512sw+512): 4 i-blocks of 128 rows; stage
                + trigger this sweep's two sub-AGs (B stages upper first)."""
                ib_order = [0, 1, 2, 3] if sw == 0 else [2, 3, 0, 1]
                ag_map = {1: 0, 3: 1} if sw == 0 else {1: 3, 3: 2}
                for i, ib in enumerate(ib_order):
                    if tps is None:
                        tp = ps.tile([P, D], f32, name=f"t_ps_{sw}_{ib}",
                                     tag="acc")
                    else:
                        tp = tps[i]
                    for dc in range(DCH):
                        nc.tensor.matmul(
                            tp[:],
                            ut_sb[sw][:, dc, P * ib : P * (ib + 1)],
                            w_sb[:, dc, :],
                            start=(dc == 0),
                            stop=(dc == DCH - 1),
                        )
                    tst = stage.tile([P, D], cdt, name=f"t_st_{sw}_{ib}",
                                     tag="st")
                    nc.vector.tensor_copy(tst[:], tp[:])
                    row0 = D * sw + P * ib
                    nc.sync.dma_start(t_in[row0 : row0 + P, :], tst[:])
                    if i in ag_map:
                        t_subag(ag_map[i])

            NB = NCH // SB  # 16 strip batches per sweep

            # ---- PE stream: pure matmuls start to finish
            sweep1(0, accsA, 0, NB, split_first=True)
            drain_u(0, accsA)
            # head of sweep B hides sweep A's drain latency before step2A
            sweep1(1, accsB, 0, 2)
            step2(0)
            sweep1(1, accsB, 2, NB)
            drain_u(1, accsB)

            # PSUM rotation: allocate step2B's tiles, then phase-3
            # accumulators — accs3[0..3] land on step2A's banks (free
            # early), accs3[4..7] on step2B's.  Seed 0..3 before step2B's
            # instructions so only 4..7 wait on its staging.
            tps1 = [ps.tile([P, D], f32, name=f"t_ps_1_{i}", tag="acc")
                    for i in range(4)]
            accs3 = [
                ps.tile([P, D], f32, name=f"acc3_{nt}", tag="acc")
                for nt in range(RCH)
            ]
            for nt in range(4):
                nc.vector.tensor_copy(accs3[nt][:], bsb[:])
            step2(1, tps1)
            for nt in range(4, RCH):
                nc.scalar.copy(accs3[nt][:], bsb[:])

            # ---- phase 3: t gather consumption + band matmuls.  t_sb
            # loads ride the otherwise-idle gpsimd queue, h-half-major so
            # the first consumed pair of each group lands after two loads.
            t_sb = bigB.tile([P, NCH, D], cdt, name="t_sb", tag="bigB")
            t_sb_r = t_sb[:].rearrange("p (c r) d -> p r c d", r=8)
            for g in GORDER:
                for h in range(2):
                    for u in range(2):
                        nc.gpsimd.dma_start(
                            t_sb_r[:, 2 * g + u, 4 * h : 4 * h + 4, :],
                            t_outs[g].rearrange(
                                "(c q p) d -> p c q d", p=P, q=2
                            )[:, 4 * h : 4 * h + 4, u, :],
                        )
            if DEBUG:
                for j in range(NCH):
                    nc.sync.dma_start(
                        t_dump[P * j : P * (j + 1), :], t_sb[:, j, :]
                    )

            HI_PAIRS = [e for e in PAIRS if e[0] >= KCH]
            LO_PAIRS = [e for e in PAIRS if e[0] < KCH]
            stash = [
                stashp.tile([P, D], f32, name=f"hst_{nt}", tag=f"hst{nt}")
                for nt in range(RCH)
            ]
            for idx, (j, g, c) in enumerate(HI_PAIRS):
                jj = j - KCH
                strip = strips3.tile([P, 2, ROWS], cdt, name=f"rh_{j}",
                                     tag="strip3")
                nc.sync.dma_start(
                    strip[:].rearrange("p a b -> p (a b)"), s2p[jj // 2]
                )
                for u in range(2):
                    for nt in range(RCH):
                        nc.tensor.matmul(
                            accs3[nt][:],
                            strip[:, u, P * nt : P * (nt + 1)],
                            t_sb[:, j + u, :],
                            start=False,
                            stop=(idx == len(HI_PAIRS) - 1 and u == 1),
                        )
            # stash = acc (= hi + bias; softmax folded into s2 on host),
            # then re-seed for the low band right behind the stash read;
            # split vector/scalar (gpsimd cannot access PSUM)
            for nt in range(4):
                nc.vector.tensor_copy(stash[nt][:], accs3[nt][:])
                nc.vector.tensor_copy(accs3[nt][:], bsb[:])
            for nt in range(4, RCH):
                nc.scalar.copy(stash[nt][:], accs3[nt][:])
                nc.scalar.copy(accs3[nt][:], bsb[:])
            # relu the stash in place (during the low band):
            # relu(max(u,v)) == max(relu(u), relu(v))
            for nt in range(RCH):
                nc.scalar.activation(
                    stash[nt][:], stash[nt][:],
                    mybir.ActivationFunctionType.Relu,
                )

            # low band: strip-major except the last NTAIL strips
            for j, g, c in LO_PAIRS[:-NTAIL]:
                strip = strips3.tile([P, 2, ROWS], cdt, name=f"rl_{j}",
                                     tag="strip3")
                nc.sync.dma_start(
                    strip[:].rearrange("p a b -> p (a b)"), s0p[j // 2]
                )
                for u in range(2):
                    for nt in range(RCH):
                        nc.tensor.matmul(
                            accs3[nt][:],
                            strip[:, u, P * nt : P * (nt + 1)],
                            t_sb[:, j + u, :],
                            start=False,
                            stop=False,
                        )
            # last NTAIL strips nt-major: accumulators complete staggered
            # (2.1us apart) so relu/max/store pipeline behind the matmuls
            tail = []
            for j, g, c in LO_PAIRS[-NTAIL:]:
                strip = strips3.tile([P, 2, ROWS], cdt, name=f"rt_{j}",
                                     tag="strip3")
                nc.sync.dma_start(
                    strip[:].rearrange("p a b -> p (a b)"), s0p[j // 2]
                )
                tail.append((j, strip))
            for nt in range(RCH):
                for ti, (j, strip) in enumerate(tail):
                    for u in range(2):
                        nc.tensor.matmul(
                            accs3[nt][:],
                            strip[:, u, P * nt : P * (nt + 1)],
                            t_sb[:, j + u, :],
                            start=False,
                            stop=(ti == len(tail) - 1 and u == 1),
                        )
                lo = epi.tile([P, D], f32, name=f"elo_{nt}", tag="elo")
                nc.scalar.activation(
                    lo[:], accs3[nt][:], mybir.ActivationFunctionType.Relu,
                )
                osb = epi.tile([P, D], f32, name=f"osb_{nt}", tag="osb")
                nc.vector.tensor_tensor(
                    osb[:], lo[:], stash[nt][:], mybir.AluOpType.max
                )
                row0 = P * nt
                nc.sync.dma_start(out[row0 : row0 + P, :], osb[:])

    nc.compile()
    return nc


def _get_nc():
    if "nc" not in _CACHE:
        _CACHE["nc"] = _build_nc()
    return _CACHE["nc"]


def _shard_inputs(x, weights, alpha, bias, s0, s1, s2, s3):
    import ml_dtypes

    cnp = ml_dtypes.bfloat16

    def prep(a, scale=None):  # transpose (+ scale) + cast, C-contiguous
        t = a.T if scale is None else a.T * scale
        return np.ascontiguousarray(t).astype(cnp, copy=False)

    # softmax(alpha) folded into the low/high band matrices host-side
    af = np.asarray(alpha, dtype=np.float64)
    e = np.exp(af - af.max())
    a_sm = (e / e.sum()).astype(np.float32)

    seedb = np.ascontiguousarray(
        np.broadcast_to(np.asarray(bias, dtype=np.float32)[None, :], (P, D))
    )
    w_p = np.ascontiguousarray(weights).astype(cnp, copy=False)
    # xp[b, p, (j2 d)] = x[128*(2b+j2)+p, d]: 2KB contiguous per partition
    x_p = np.ascontiguousarray(
        x.astype(cnp, copy=False).reshape(NCH // 2, 2, P, D).transpose(0, 2, 1, 3)
        .reshape(NCH // 2, P, 2 * D)
    )

    def pack_sweeps(t):  # t: [n, i] transposed s-matrix -> [2, NB, P, SB*D]
        n = t.shape[0]
        a = t.reshape(n // (SB * P), SB, P, 2, D).transpose(3, 0, 2, 1, 4)
        return np.ascontiguousarray(a.reshape(2, n // (SB * P), P, SB * D))

    def pack_pairs(t):  # t: [n, i] -> [n/256, P, 2*i]
        n, i = t.shape
        a = t.reshape(n // (2 * P), 2, P, i).transpose(0, 2, 1, 3)
        return np.ascontiguousarray(a.reshape(n // (2 * P), P, 2 * i))

    in_maps = []
    for c in range(N_CORES):
        r0, r1 = ROWS * c, ROWS * (c + 1)
        # S = concat(s1, s3) rows; core c owns rows [r0, r1)
        if r1 <= K:
            s_rows = s1[r0:r1]
        elif r0 >= K:
            s_rows = s3[r0 - K : r1 - K]
        else:  # straddles the boundary (not the case for these shapes)
            s_rows = np.concatenate([s1[r0:], s3[: r1 - K]], axis=0)
        in_maps.append(
            {
                "xp": x_p,
                "w": w_p,
                "seedb": seedb,
                "stp": pack_sweeps(prep(s_rows)),
                "s0p": pack_pairs(prep(s0[r0:r1], a_sm[0])),
                "s2p": pack_pairs(prep(s2[r0:r1], a_sm[1])),
            }
        )
    return in_maps


def kernel(x, weights, alpha, bias, s0, s1, s2, s3, _trace=False):
    from concourse.bass_utils import run_bass_kernel_spmd

    nc = _get_nc()
    in_maps = _shard_inputs(
        np.asarray(x), np.asarray(weights), np.asarray(alpha), np.asarray(bias),
        np.asarray(s0), np.asarray(s1), np.asarray(s2), np.asarray(s3),
    )
    kwargs = {}
    if _trace:
        run_bass_kernel_spmd(nc, in_maps, core_ids=list(range(N_CORES)))
        kwargs = dict(trace=True, trace_cores=list(range(N_CORES)))
    r = run_bass_kernel_spmd(nc, in_maps, core_ids=list(range(N_CORES)), **kwargs)
    full = np.concatenate([res["out"] for res in r.results], axis=0)
    if _trace:
        return full, r
    return full


# revision 17
# speedup vs baseline: 1.0367x; 1.0367x over previous
"""SpGAT_Conv Trainium2 kernel: 8-core SPMD spectral GNN conv.

Math (reference):
    a = softmax(alpha)
    pre = x @ W                                   [N, D]
    out_low  = s0 @ (a0 * (s1 @ pre))             [N, D]
    out_high = s2 @ (a1 * (s3 @ pre))             [N, D]
    out = relu(max(out_low, out_high) + bias)

Re-association: t = S @ (x @ W) == (S @ x) @ W with S = concat(s1, s3).
Row-sharding t's rows across 8 cores makes the x@W work perfectly sharded
too (it rides on each core's own 1024 rows of u = S_c @ x) instead of being
replicated, cutting per-core PE work from 1280 to 1056 big matmuls:

    step 1: u_c^T = x^T S_c^T accumulated over n-chunks; stationary = x
            chunks (natural layout), moving = S_c^T strips.  Two sweeps of
            512 i-columns each (4 PSUM banks per sweep).
    step 2: t_c = u_c @ W via u^T slices stationary, W moving (32 matmuls);
            each sweep's 512 t rows are staged + AllGathered in two
            sub-collectives (4 total, order 0,1,3,2) while later compute
            runs.
    phase 3: out_c = relu(max(s0_c@t1 + bias, s2_c@t3 + bias)), high band
             then low band over ARRIVAL-ordered t chunks; softmax(alpha)
             is folded into s0/s2 on the host, bias is passed
             pre-broadcast [128, D] and seeded into PSUM, so the epilogue
             per strip is one ACT relu + one DVE max.  The last four
             low-band strips run nt-major so the 8 accumulators complete
             staggered and the epilogue pipelines behind the matmuls.

Schedule notes:
  - PE warm-up junk matmuls keep the HAM activity window busy from
    ~6.5us so the first real matmuls aren't clock-gated.
  - Steady-state matmul rate is ~263ns (chip P0 power downclock to
    ~2.0GHz under sustained full-chip load; pure-PE microbenches run at
    216ns but PE+DMA+CC tips the power budget).  The schedule aims for a
    gapless PE stream rather than fighting the clock.
  - strips3 bufs=5 gives phase 3 a ~21us strip cushion so the AllGather
    ring-traffic windows (which degrade concurrent DMA supply) don't
    starve the PE.

DMA discipline: DMA_DIRECT2D executes synchronously on the issuing
engine's queue AND all queues share one small DMA-completion semaphore
pool, so a blocked DMA (or a semaphore-reset rendezvous with one) stalls
unrelated queues.  Mitigations:
  - Sync queue: only matmul-pacing strip loads, t_in stores, the out
    stores (tail-only) — never anything collective-gated.
  - x/w bulk loads ride scalar + gpsimd as FEW BIG block DMAs (small
    DMA counts keep semaphore epochs from recycling).
  - AllGather-gated t_sb loads ride the otherwise-idle gpsimd queue.

All big operands are host-cast to bf16 (full PE rate) with fp32 PSUM
accumulation; s-matrices are host-transposed so the contraction dim lands
on SBUF partitions with contiguous DMAs.  x needs no transpose in this
formulation.
"""

import os

import numpy as np

N_CORES = 8
N = 8192
K = 2048
NK = N - K          # 6144
D = 512
ROWS = N // N_CORES  # 1024 rows per core
P = 128
RCH = ROWS // P      # 8  (row chunks per core / output strips)
NCH = N // P         # 64 (contraction chunks over full N)
KCH = K // P         # 16 (low-band chunks; high band = NCH - KCH = 48)
DCH = D // P         # 4  (depth chunks)
NSUB = 4             # sub-AllGathers for t
SUBR = ROWS // NSUB  # 256 rows per rank per sub-AG
SB = 4               # n-chunks per sweep-strip DMA batch
NWARM = 8            # PE warm-up junk matmuls
NTAIL = 4            # low-band strips run nt-major for the epilogue

DEBUG = os.environ.get("SPGAT_DEBUG", "0") == "1"

_CACHE = {}

# t-chunk arrival order: sub-AG g delivers, for every rank c, t rows
# [1024c + 256g, 1024c + 256(g+1)) = global chunks 8c + 2g + {0,1}.
# Gathers are triggered in order 0, 1, 3, 2 (sweep B stages its upper rows
# first), so consume in that order too.  Phase 3 iterates PAIRS (both u's
# of one (g, c)) so each pair is one strip DMA.
GORDER = [0, 1, 3, 2]
PAIRS = [(8 * c + 2 * g, g, c) for g in GORDER for c in range(N_CORES)]


def _build_nc():
    import concourse.mybir as mybir
    import concourse.tile as tile
    from concourse import bacc

    f32 = mybir.dt.float32
    bf16 = mybir.dt.bfloat16
    cdt = bf16

    nc = bacc.Bacc(
        "TRN2", target_bir_lowering=False, debug=False, num_devices=N_CORES
    )

    # all big operands host-packed so every DMA reads 2KB+ contiguous
    # per-partition lines (max descriptor efficiency)
    xp = nc.dram_tensor("xp", [NCH // 2, P, 2 * D], cdt,
                        kind="ExternalInput").ap()
    w = nc.dram_tensor("w", [D, D], cdt, kind="ExternalInput").ap()
    seedb = nc.dram_tensor("seedb", [P, D], f32, kind="ExternalInput").ap()
    stp = nc.dram_tensor("stp", [2, NCH // SB, P, SB * D], cdt,
                         kind="ExternalInput").ap()
    s0p = nc.dram_tensor("s0p", [KCH // 2, P, 2 * ROWS], cdt,
                         kind="ExternalInput").ap()
    s2p = nc.dram_tensor("s2p", [(NCH - KCH) // 2, P, 2 * ROWS], cdt,
                         kind="ExternalInput").ap()
    out = nc.dram_tensor("out", [ROWS, D], f32, kind="ExternalOutput").ap()
    if DEBUG:
        t_dump = nc.dram_tensor("t_dump", [N, D], cdt, kind="ExternalOutput").ap()

    groups = [list(range(N_CORES))]

    with tile.TileContext(nc) as tc:
        with (
            tc.tile_pool(name="const", bufs=1) as const,
            tc.tile_pool(name="bigA", bufs=1) as bigA,
            tc.tile_pool(name="bigB", bufs=1) as bigB,
            tc.tile_pool(name="strips1", bufs=4) as strips1,
            tc.tile_pool(name="strips3", bufs=5) as strips3,
            tc.tile_pool(name="stage", bufs=4) as stage,
            tc.tile_pool(name="epi", bufs=2) as epi,
            tc.tile_pool(name="stash", bufs=1) as stashp,
            tc.tile_pool(name="ps", bufs=8, space="PSUM") as ps,
            tc.tile_pool(name="dram", bufs=1, space="DRAM") as dram,
        ):
            # ---- collective warm-up: absorb CC startup + skew.  No input
            # deps so the trigger is the very first gpsimd instruction.
            warm_in = dram.tile([8, 8], f32, name="warm_in")
            warm_out = dram.tile([64, 8], f32, name="warm_out", addr_space="Shared")
            nc.gpsimd.collective_compute(
                "AllGather",
                mybir.AluOpType.bypass,
                replica_groups=groups,
                ins=[warm_in.opt()],
                outs=[warm_out.opt()],
            )

            # ---- PSUM allocation order fixes the 8-bank rotation; all
            # tiles are one 2KB bank.
            accsA = [ps.tile([P, D], f32, name=f"uA_{dc}", tag="acc")
                     for dc in range(DCH)]
            accsB = [ps.tile([P, D], f32, name=f"uB_{dc}", tag="acc")
                     for dc in range(DCH)]

            # ---- PE warm-up junk matmuls (HAM activity) on memset tiles;
            # they accumulate into accsA[0] with start=True and the first
            # real matmul (j == 0, start=True) resets the bank.
            wj = const.tile([P, P], cdt, name="wj")
            mj = const.tile([P, D], cdt, name="mj")
            nc.vector.memset(wj[:], 0.0)
            nc.vector.memset(mj[:], 0.0)
            for _ in range(NWARM):
                nc.tensor.matmul(accsA[0][:], wj[:], mj[:], start=True, stop=True)

            # ---- input DMAs.  Sync: x chunks 0,1 singly (j=0 matmuls
            # fire asap), then only strip loads.  x bulk rides scalar +
            # gpsimd as few big paced blocks; w + bias seed on scalar.
            x_sb = bigA.tile([P, NCH, D], cdt, name="x_sb", tag="bigA")
            nc.sync.dma_start(x_sb[:, 0, :], xp[0][:, :D])
            nc.sync.dma_start(x_sb[:, 1, :], xp[0][:, D:])
            blocks = [(1, 2), (2, 3), (3, 4), (4, 6), (6, 8), (8, 10),
                      (10, 12), (12, 16), (16, 20), (20, 24), (24, 32)]
            for b0, b1 in blocks:
                nc.scalar.dma_start(
                    x_sb[:, 2 * b0 : 2 * b1, :].rearrange("p c d -> p (c d)"),
                    xp[b0:b1].rearrange("b p a -> p b a"),
                )
            w_sb = const.tile([P, DCH, D], cdt, name="w_sb")
            nc.scalar.dma_start(w_sb[:], w.rearrange("(c p) d -> p c d", p=P))
            bsb = const.tile([P, D], f32, name="bsb")
            nc.scalar.dma_start(bsb[:], seedb[:])

            # ---- steps 1+2 infrastructure
            t_in = dram.tile([ROWS, D], cdt, name="t_in")
            t_outs = [
                dram.tile([SUBR * N_CORES, D], cdt, name=f"t_out{g}",
                          addr_space="Shared")
                for g in range(NSUB)
            ]

            def t_subag(g):
                nc.gpsimd.collective_compute(
                    "AllGather",
                    mybir.AluOpType.bypass,
                    replica_groups=groups,
                    ins=[t_in[SUBR * g : SUBR * (g + 1), :].opt()],
                    outs=[t_outs[g].opt()],
                )

            ut_sb = [
                const.tile([P, DCH, D], cdt, name=f"ut{sw}") for sw in range(2)
            ]

            def sweep1(sw, accs, b0, b1, split_first=False):
                """step-1 matmuls for strip batches [b0, b1) of sweep sw;
                each batch is SB n-chunks in one contiguous-packed DMA."""
                for bk in range(b0, b1):
                    j0 = SB * bk
                    strip = strips1.tile([P, SB, D], cdt, name=f"s{sw}_{bk}",
                                         tag="strip")
                    if split_first and bk == b0:
                        for j2 in range(SB):
                            nc.sync.dma_start(
                                strip[:, j2, :],
                                stp[sw, bk][:, D * j2 : D * (j2 + 1)],
                            )
                    else:
                        nc.sync.dma_start(
                            strip[:].rearrange("p a b -> p (a b)"),
                            stp[sw, bk],
                        )
                    for j2 in range(SB):
                        j = j0 + j2
                        for dc in range(DCH):
                            nc.tensor.matmul(
                                accs[dc][:],
                                x_sb[:, j, P * dc : P * (dc + 1)],
                                strip[:, j2, :],
                                start=(j == 0),
                                stop=(j == NCH - 1),
                            )

            def drain_u(sw, accs):
                # split across DVE and ACT so the banks free ~2x faster
                for dc in range(2):
                    nc.vector.tensor_copy(ut_sb[sw][:, dc, :], accs[dc][:])
                for dc in range(2, DCH):
                    nc.scalar.copy(ut_sb[sw][:, dc, :], accs[dc][:])

            def step2(sw, tps=None):
                """t rows [512sw, 512sw+512): 4 i-blocks of 128 rows; stage
                + trigger this sweep's two sub-AGs (B stages upper first)."""
                ib_order = [0, 1, 2, 3] if sw == 0 else [2, 3, 0, 1]
                ag_map = {1: 0, 3: 1} if sw == 0 else {1: 3, 3: 2}
                for i, ib in enumerate(ib_order):
                    if tps is None:
                        tp = ps.tile([P, D], f32, name=f"t_ps_{sw}_{ib}",
                                     tag="acc")
                    else:
                        tp = tps[i]
                    for dc in range(DCH):
                        nc.tensor.matmul(
                            tp[:],
                            ut_sb[sw][:, dc, P * ib : P * (ib + 1)],
                            w_sb[:, dc, :],
                            start=(dc == 0),
                            stop=(dc == DCH - 1),
                        )
                    tst = stage.tile([P, D], cdt, name=f"t_st_{sw}_{ib}",
                                     tag="st")
                    nc.vector.tensor_copy(tst[:], tp[:])
                    row0 = D * sw + P * ib
                    nc.sync.dma_start(t_in[row0 : row0 + P, :], tst[:])
                    if i in ag_map:
                        t_subag(ag_map[i])

            NB = NCH // SB  # 16 strip batches per sweep

            # ---- PE stream: pure matmuls start to finish
            sweep1(0, accsA, 0, NB, split_first=True)
            drain_u(0, accsA)
            # head of sweep B hides sweep A's drain latency before step2A
            sweep1(1, accsB, 0, 2)
            step2(0)
            sweep1(1, accsB, 2, NB)
            drain_u(1, accsB)

            # PSUM rotation: allocate step2B's tiles, then phase-3
            # accumulators — accs3[0..3] land on step2A's banks (free
            # early), accs3[4..7] on step2B's.  Seed 0..3 before step2B's
            # instructions so only 4..7 wait on its staging.
            tps1 = [ps.tile([P, D], f32, name=f"t_ps_1_{i}", tag="acc")
                    for i in range(4)]
            accs3 = [
                ps.tile([P, D], f32, name=f"acc3_{nt}", tag="acc")
                for nt in range(RCH)
            ]
            for nt in range(4):
                nc.vector.tensor_copy(accs3[nt][:], bsb[:])
            step2(1, tps1)
            for nt in range(4, RCH):
                nc.scalar.copy(accs3[nt][:], bsb[:])

            # ---- phase 3: t gather consumption + band matmuls.  t_sb
            # loads ride the otherwise-idle gpsimd queue, h-half-major so
            # the first consumed pair of each group lands after two loads.
            t_sb = bigB.tile([P, NCH, D], cdt, name="t_sb", tag="bigB")
            t_sb_r = t_sb[:].rearrange("p (c r) d -> p r c d", r=8)
            for g in GORDER:
                for h in range(2):
                    for u in range(2):
                        nc.gpsimd.dma_start(
                            t_sb_r[:, 2 * g + u, 4 * h : 4 * h + 4, :],
                            t_outs[g].rearrange(
                                "(c q p) d -> p c q d", p=P, q=2
                            )[:, 4 * h : 4 * h + 4, u, :],
                        )
            if DEBUG:
                for j in range(NCH):
                    nc.sync.dma_start(
                        t_dump[P * j : P * (j + 1), :], t_sb[:, j, :]
                    )

            HI_PAIRS = [e for e in PAIRS if e[0] >= KCH]
            LO_PAIRS = [e for e in PAIRS if e[0] < KCH]
            stash = [
                stashp.tile([P, D], f32, name=f"hst_{nt}", tag=f"hst{nt}")
                for nt in range(RCH)
            ]
            for idx, (j, g, c) in enumerate(HI_PAIRS):
                jj = j - KCH
                strip = strips3.tile([P, 2, ROWS], cdt, name=f"rh_{j}",
                                     tag="strip3")
                nc.sync.dma_start(
                    strip[:].rearrange("p a b -> p (a b)"), s2p[jj // 2]
                )
                for u in range(2):
                    for nt in range(RCH):
                        nc.tensor.matmul(
                            accs3[nt][:],
                            strip[:, u, P * nt : P * (nt + 1)],
                            t_sb[:, j + u, :],
                            start=False,
                            stop=(idx == len(HI_PAIRS) - 1 and u == 1),
                        )
            # stash = acc (= hi + bias; softmax folded into s2 on host),
            # then re-seed for the low band right behind the stash read;
            # split vector/scalar (gpsimd cannot access PSUM)
            for nt in range(4):
                nc.vector.tensor_copy(stash[nt][:], accs3[nt][:])
                nc.vector.tensor_copy(accs3[nt][:], bsb[:])
            for nt in range(4, RCH):
                nc.scalar.copy(stash[nt][:], accs3[nt][:])
                nc.scalar.copy(accs3[nt][:], bsb[:])
            # relu the stash in place (during the low band):
            # relu(max(u,v)) == max(relu(u), relu(v))
            for nt in range(RCH):
                nc.scalar.activation(
                    stash[nt][:], stash[nt][:],
                    mybir.ActivationFunctionType.Relu,
                )

            # low band: strip-major except the last NTAIL strips
            for j, g, c in LO_PAIRS[:-NTAIL]:
                strip = strips3.tile([P, 2, ROWS], cdt, name=f"rl_{j}",
                                     tag="strip3")
                nc.sync.dma_start(
                    strip[:].rearrange("p a b -> p (a b)"), s0p[j // 2]
                )
                for u in range(2):
                    for nt in range(RCH):
                        nc.tensor.matmul(
                            accs3[nt][:],
                            strip[:, u, P * nt : P * (nt + 1)],
                            t_sb[:, j + u, :],
                            start=False,
                            stop=False,
                        )
            # last NTAIL strips nt-major: accumulators complete staggered
            # (2.1us apart) so relu/max/store pipeline behind the matmuls
            tail = []
            for j, g, c in LO_PAIRS[-NTAIL:]:
                strip = strips3.tile([P, 2, ROWS], cdt, name=f"rt_{j}",
                                     tag="strip3")
                nc.sync.dma_start(
                    strip[:].rearrange("p a b -> p (a b)"), s0p[j // 2]
                )
                tail.append((j, strip))
            for nt in range(RCH):
                for ti, (j, strip) in enumerate(tail):
                    for u in range(2):
                        nc.tensor.matmul(
                            accs3[nt][:],
                            strip[:, u, P * nt : P * (nt + 1)],
                            t_sb[:, j + u, :],
                            start=False,
                            stop=(ti == len(tail) - 1 and u == 1),
                        )
                lo = epi.tile([P, D], f32, name=f"elo_{nt}", tag="elo")
                nc.scalar.activation(
                    lo[:], accs3[nt][:], mybir.ActivationFunctionType.Relu,
                )
                osb = epi.tile([P, D], f32, name=f"osb_{nt}", tag="osb")
                nc.vector.tensor_tensor(
                    osb[:], lo[:], stash[nt][:], mybir.AluOpType.max
                )
                row0 = P * nt
                nc.sync.dma_start(out[row0 : row0 + P, :], osb[:])

    nc.compile()
    return nc


def _get_nc():
    if "nc" not in _CACHE:
        _CACHE["nc"] = _build_nc()
    return _CACHE["nc"]


def _shard_inputs(x, weights, alpha, bias, s0, s1, s2, s3):
    import ml_dtypes

    cnp = ml_dtypes.bfloat16

    def prep(a, scale=None):  # transpose (+ scale) + cast, C-contiguous
        t = a.T if scale is None else a.T * scale
        return np.ascontiguousarray(t).astype(cnp, copy=False)

    # softmax(alpha) folded into the low/high band matrices host-side
    af = np.asarray(alpha, dtype=np.float64)
    e = np.exp(af - af.max())
    a_sm = (e / e.sum()).astype(np.float32)

    seedb = np.ascontiguousarray(
        np.broadcast_to(np.asarray(bias, dtype=np.float32)[None, :], (P, D))
    )
    w_p = np.ascontiguousarray(weights).astype(cnp, copy=False)
    # xp[b, p, (j2 d)] = x[128*(2b+j2)+p, d]: 2KB contiguous per partition
    x_p = np.ascontiguousarray(
        x.astype(cnp, copy=False).reshape(NCH // 2, 2, P, D).transpose(0, 2, 1, 3)
        .reshape(NCH // 2, P, 2 * D)
    )

    def pack_sweeps(t):  # t: [n, i] transposed s-matrix -> [2, NB, P, SB*D]
        n = t.shape[0]
        a = t.reshape(n // (SB * P), SB, P, 2, D).transpose(3, 0, 2, 1, 4)
        return np.ascontiguousarray(a.reshape(2, n // (SB * P), P, SB * D))

    def pack_pairs(t):  # t: [n, i] -> [n/256, P, 2*i]
        n, i = t.shape
        a = t.reshape(n // (2 * P), 2, P, i).transpose(0, 2, 1, 3)
        return np.ascontiguousarray(a.reshape(n // (2 * P), P, 2 * i))

    in_maps = []
    for c in range(N_CORES):
        r0, r1 = ROWS * c, ROWS * (c + 1)
        # S = concat(s1, s3) rows; core c owns rows [r0, r1)
        if r1 <= K:
            s_rows = s1[r0:r1]
        elif r0 >= K:
            s_rows = s3[r0 - K : r1 - K]
        else:  # straddles the boundary (not the case for these shapes)
            s_rows = np.concatenate([s1[r0:], s3[: r1 - K]], axis=0)
        in_maps.append(
            {
                "xp": x_p,
                "w": w_p,
                "seedb": seedb,
                "stp": pack_sweeps(prep(s_rows)),
                "s0p": pack_pairs(prep(s0[r0:r1], a_sm[0])),
                "s2p": pack_pairs(prep(s2[r0:r1], a_sm[1])),
            }
        )
    return in_maps


def kernel(x, weights, alpha, bias, s0, s1, s2, s3, _trace=False):
    from concourse.bass_utils import run_bass_kernel_spmd

    nc = _get_nc()
    in_maps = _shard_inputs(
        np.asarray(x), np.asarray(weights), np.asarray(alpha), np.asarray(bias),
        np.asarray(s0), np.asarray(s1), np.asarray(s2), np.asarray(s3),
    )
    kwargs = {}
    if _trace:
        run_bass_kernel_spmd(nc, in_maps, core_ids=list(range(N_CORES)))
        kwargs = dict(trace=True, trace_cores=list(range(N_CORES)))
    r = run_bass_kernel_spmd(nc, in_maps, core_ids=list(range(N_CORES)), **kwargs)
    full = np.concatenate([res["out"] for res in r.results], axis=0)
    if _trace:
        return full, r
    return full
